# revision 38
# baseline (speedup 1.0000x reference)
"""Trainium2 Bass kernel for nn_Net_84782654423525 (GNN message passing + LSTM).

Strategy (8 NeuronCores, dst-sharded nodes):
  Launch A (mpnn1): per core, HBM-source dma_gather of rsg1-prescaled X
    rows for edges whose dst it owns. Rows are fp16 [128] viewed as
    uint32 [64] (elem_size=64: halves the per-descriptor DMA cost, which
    scales with element count, not bytes). transpose=False output is
    [128 dst, K slots, 64] per degree-sorted 128-node group with fixed
    padded K. Slot-sum split across the otherwise-idle PE (identity-
    matmul PSUM accumulate) and DVE (fp16 tree adds); relu on Act ->
    h1 shard (token layout, fp16) to HBM.
  Host: reassemble full h1 (unpermute degree-sorted rows), apply the
    mean 1/cnt scale and bet1, build the rsg2-prescaled uint32 gather
    source for mpnn2; fold bet2 into per-timestep LSTM bias b1.
  Launch B (mpnn2 + 2-layer LSTM + dense): same uint32 HBM gathers ->
    DVE tree-sum (PE assists during the t=0 prologue) -> relu on GPSIMD
    -> h2 token tile; un-permute + transpose via one SBUF-source gather;
    LSTM over [h1;h2] with fp16 matmuls (features-on-partitions), fp32
    PSUM, gate activations on Act, the two independent elementwise muls
    on GPSIMD and the c-update chain on DVE; final dense + ReLU.

Engine balance per launch (cost model): A ~226us = max(DMA gathers ~213,
PE+DVE split tree ~110 each); B ~461us = max(PE 399 LSTM, Act 376 gates,
DVE 350 tree+chain, Pool 340 descgen+relu/mul offload).
"""

import os
import sys
from contextlib import ExitStack

import numpy as np

sys.path.insert(0, "/opt/trn_rl_repo")

import concourse.bacc as bacc
import concourse.tile as tile
from concourse import mybir
from concourse.bass_utils import run_bass_kernel_spmd

HDT = mybir.dt.float16
F32 = mybir.dt.float32
I16 = mybir.dt.int16
U32 = mybir.dt.uint32
AF = mybir.ActivationFunctionType
EPS = 1e-3
NCORES = 8

PROFILE = bool(int(os.environ.get("KERNEL_PROFILE", "0")))
LAST_STATS = {}

try:  # trace=True requires antenv.axon_hooks; fall back gracefully
    from antenv.axon_hooks import get_axon_ntff_profile_hook  # noqa: F401
except Exception:
    PROFILE = False


# ---------------------------------------------------------------- host prep

def _pack_idx_blocks(stream, ks):
    """Per-group idx blocks packed for dma_gather: idx i of a block lives at
    [i % 16, i // 16]; blocks concatenated along cols; tiled to 128 rows."""
    blocks = []
    off = 0
    for k in ks:
        n = 128 * int(k)
        s = stream[off : off + n]
        blocks.append(s.reshape(n // 16, 16).T)
        off += n
    m = np.concatenate(blocks, axis=1)  # [16, L/16]
    return np.ascontiguousarray(np.tile(m, (8, 1))).astype(np.int16)


def _plan_t(src, dst, n, ncores, shp, pad_tok):
    """Edge plan for one timestep with degree-sorted node groups (tight K).

    Nodes of each core's shard are permuted into degree-descending order so
    that per-group max degree (the padded slot count K) is near the group's
    degree quantile across all cores. Returns (K[NG], streams per core,
    cnt per core (permuted order), perm per core)."""
    sh = n // ncores
    ng = shp // 128
    per_core = []
    for c in range(ncores):
        m = (dst >= c * sh) & (dst < (c + 1) * sh)
        dl = (dst[m] - c * sh).astype(np.int64)
        sl = src[m].astype(np.int64)
        order = np.argsort(dl, kind="stable")
        dl = dl[order]
        sl = sl[order]
        cnt = np.bincount(dl, minlength=sh)
        perm = np.argsort(-cnt, kind="stable")  # natural ids, deg-desc order
        pos_of = np.empty(sh, np.int64)
        pos_of[perm] = np.arange(sh)
        per_core.append((dl, sl, cnt, perm, pos_of))
    K = np.full(ng, 2, np.int64)
    for dl, sl, cnt, perm, pos_of in per_core:
        cp = np.zeros(shp, np.int64)
        cp[:sh] = cnt[perm]
        K = np.maximum(K, cp.reshape(ng, 128).max(1))
    K = np.maximum(K, 2)
    base = np.concatenate([[0], np.cumsum(128 * K)])
    L = int(base[-1])
    streams, cnts, perms = [], [], []
    for dl, sl, cnt, perm, pos_of in per_core:
        stream = np.full(L, pad_tok, np.int64)
        starts = np.concatenate([[0], np.cumsum(cnt)])
        j = np.arange(dl.size) - starts[dl]
        p = pos_of[dl]  # permuted position of each edge's dst
        pos = base[p // 128] + j * 128 + (p % 128)
        stream[pos] = sl
        streams.append(stream)
        cnts.append(cnt)
        perms.append(perm)
    return K, streams, cnts, perms


# ---------------------------------------------------------- device builders

def _tree(nc, gt, k):
    """In-place slot-dim tree sum of gt (uint32 tile [128, k, f//2], treated
    as fp16 [128, k, f]); result in gt[:, 0, :].bitcast(f16)."""
    v = lambda a, b: gt[:, a:b, :].bitcast(HDT)
    cur = k
    while cur > 1:
        if cur % 2:
            nc.vector.tensor_add(v(0, 1), v(0, 1), v(cur - 1, cur))
            cur -= 1
        h = cur // 2
        nc.vector.tensor_add(v(0, h), v(0, h), v(h, cur))
        cur = h


def _emit_mpnn(nc, pools, src_d, idxt, Ks, t, f, consume):
    """HBM-source uint32 gather (elem=f//2 x 4B = one fp16 feature row per
    index) for one timestep. Calls consume(g, gt) per group right after its
    gather; gt is uint32 [128, kg, f//2] holding fp16 rows."""
    ng = len(Ks)
    e4 = f // 2  # uint32 elements per fp16 feature row
    goff = 0
    for g in range(ng):
        kg = int(Ks[g])
        gt = pools["g"].tile([128, kg, e4], U32, tag="g")
        nc.gpsimd.dma_gather(
            gt[:],
            src_d.ap()[t],
            idxt[:, goff // 16 : (goff + 128 * kg) // 16],
            128 * kg,
            128 * kg,
            e4,
            transpose=False,
            single_packet=False,
        )
        goff += 128 * kg
        consume(g, gt)


def _build_launch_a(Ks_all, w, f, nrows, shp):
    ng = shp // 128
    nc = bacc.Bacc("TRN2", target_bir_lowering=False, debug=False,
                   num_devices=NCORES)
    Ltot = int(sum(128 * K.sum() for K in Ks_all))
    xs_d = nc.dram_tensor("xs", [w, nrows, f // 2], U32, kind="ExternalInput")
    idx_d = nc.dram_tensor("idx", [128, Ltot // 16], I16, kind="ExternalInput")
    ident_d = nc.dram_tensor("ident", [128, 128], HDT, kind="ExternalInput")
    h1_d = nc.dram_tensor("h1", [w, 128, ng * f], HDT, kind="ExternalOutput")

    with tile.TileContext(nc) as tc, ExitStack() as ctx, \
            nc.allow_low_precision(reason="fp16 sums by design"):
        pools = {
            "idx": ctx.enter_context(tc.tile_pool(name="idx", bufs=2)),
            "g": ctx.enter_context(tc.tile_pool(name="g", bufs=4)),
            "hs": ctx.enter_context(tc.tile_pool(name="hs", bufs=2)),
            "w": ctx.enter_context(tc.tile_pool(name="w", bufs=1)),
            "psum": ctx.enter_context(tc.tile_pool(name="psum", bufs=6,
                                                   space="PSUM")),
        }
        identt_a = pools["w"].tile([128, 128], HDT, tag="ident")
        nc.sync.dma_start(identt_a[:], ident_d.ap()[:])
        idx_off = 0
        for t in range(w):
            Lt = int(128 * Ks_all[t].sum())
            idxt = pools["idx"].tile([128, Lt // 16], I16, tag="idx")
            nc.sync.dma_start(
                idxt[:], idx_d.ap()[:, idx_off // 16 : (idx_off + Lt) // 16])
            hs = pools["hs"].tile([128, ng, f], HDT, tag="hs")

            def consume(g, gt, hs=hs, t=t):
                # slot-sum split across the otherwise-idle PE and DVE
                kg = int(Ks_all[t][g])
                if g % 2 == 0:
                    ps = pools["psum"].tile([128, f], F32, tag="ps")
                    for j in range(kg):
                        nc.tensor.matmul(ps[:], identt_a[:],
                                         gt[:, j, :].bitcast(HDT),
                                         start=(j == 0), stop=(j == kg - 1))
                    nc.scalar.activation(hs[:, g, :], ps[:], AF.Relu)
                else:
                    _tree(nc, gt, kg)
                    nc.scalar.activation(hs[:, g, :],
                                         gt[:, 0, :].bitcast(HDT), AF.Relu)

            _emit_mpnn(nc, pools, xs_d, idxt, Ks_all[t], t, f, consume)
            idx_off += Lt
            nc.sync.dma_start(h1_d.ap()[t], hs[:])
    nc.compile()
    return nc


def _lstm_step(nc, pools, xc, ka, kb, ra, rb_, btile, hbf, cst, first, shp, ct):
    """One LSTM step. xc: (chunk0, chunk1) fp16 [128, shp] APs of x^T.
    hbf/cst: [128, 2*shp] fp16 tiles (h, c), updated in place."""
    ntile = shp // ct
    for nt in range(ntile):
        cs = slice(nt * ct, (nt + 1) * ct)
        gates = []
        for gc in range(8):
            gs = slice(gc * 128, (gc + 1) * 128)
            ps = pools["psum"].tile([128, ct], F32, tag="ps")
            nc.tensor.matmul(ps[:], ka[:, gs], xc[0][:, cs], start=True,
                             stop=False)
            nc.tensor.matmul(ps[:], kb[:, gs], xc[1][:, cs], start=False,
                             stop=first)
            if not first:
                nc.tensor.matmul(ps[:], ra[:, gs], hbf[:, nt * ct : (nt + 1) * ct],
                                 start=False, stop=False)
                nc.tensor.matmul(ps[:], rb_[:, gs],
                                 hbf[:, shp + nt * ct : shp + (nt + 1) * ct],
                                 start=False, stop=True)
            gt_ = pools["gate"].tile([128, ct], HDT, tag="gate")
            func = AF.Tanh if gc in (4, 5) else AF.Sigmoid
            nc.scalar.activation(gt_[:], ps[:], func, bias=btile[:, gc : gc + 1])
            gates.append(gt_)
        for uc in range(2):
            i_, f_, g_, o_ = gates[0 + uc], gates[2 + uc], gates[4 + uc], gates[6 + uc]
            csl = cst[:, uc * shp + nt * ct : uc * shp + (nt + 1) * ct]
            tmp = pools["tmp"].tile([128, ct], HDT, tag="tmp")
            # the two independent muls go to GPSIMD to relieve DVE; the
            # c-chain (f*c, +) stays on DVE for latency
            nc.gpsimd.tensor_mul(tmp[:], i_[:], g_[:])
            if first:
                nc.vector.tensor_copy(csl, tmp[:])
            else:
                nc.vector.tensor_mul(csl, f_[:], csl)
                nc.vector.tensor_add(csl, csl, tmp[:])
            th = pools["tmp"].tile([128, ct], HDT, tag="th")
            nc.scalar.activation(th[:], csl, AF.Tanh)
            nc.gpsimd.tensor_mul(hbf[:, uc * shp + nt * ct : uc * shp + (nt + 1) * ct],
                                 o_[:], th[:])


def _build_launch_b(Ks_all, w, f, nrows, shp, u4):
    ng = shp // 128
    nc = bacc.Bacc("TRN2", target_bir_lowering=False, debug=False,
                   num_devices=NCORES)
    Ltot = int(sum(128 * K.sum() for K in Ks_all))
    ct = min(512, shp)
    hsc_d = nc.dram_tensor("hsc", [w, nrows, f // 2], U32, kind="ExternalInput")
    idx_d = nc.dram_tensor("idx", [128, Ltot // 16], I16, kind="ExternalInput")
    h1t_d = nc.dram_tensor("h1t", [w, 128, shp], HDT, kind="ExternalInput")
    k1_d = nc.dram_tensor("k1", [256, u4], HDT, kind="ExternalInput")
    r1_d = nc.dram_tensor("r1", [256, u4], HDT, kind="ExternalInput")
    k2_d = nc.dram_tensor("k2", [256, u4], HDT, kind="ExternalInput")
    r2_d = nc.dram_tensor("r2", [256, u4], HDT, kind="ExternalInput")
    ident_d = nc.dram_tensor("ident", [128, 128], HDT, kind="ExternalInput")
    b1_d = nc.dram_tensor("b1", [128, w * 8], F32, kind="ExternalInput")
    b2_d = nc.dram_tensor("b2", [128, 8], F32, kind="ExternalInput")
    wd_d = nc.dram_tensor("wd", [128, 2], HDT, kind="ExternalInput")
    bd_d = nc.dram_tensor("bd", [1, 1], F32, kind="ExternalInput")
    pidx_d = nc.dram_tensor("pinv", [w, 128, shp // 16], I16,
                            kind="ExternalInput")
    y_d = nc.dram_tensor("y", [1, shp], F32, kind="ExternalOutput")

    with tile.TileContext(nc) as tc, ExitStack() as ctx, \
            nc.allow_low_precision(reason="fp16 state/tree by design"):
        pools = {
            "idx": ctx.enter_context(tc.tile_pool(name="idx", bufs=2)),
            "g": ctx.enter_context(tc.tile_pool(name="g", bufs=4)),
            "misc": ctx.enter_context(tc.tile_pool(name="misc", bufs=2)),
            "w": ctx.enter_context(tc.tile_pool(name="w", bufs=1)),
            "state": ctx.enter_context(tc.tile_pool(name="state", bufs=1)),
            "gate": ctx.enter_context(tc.tile_pool(name="gate", bufs=10)),
            "tmp": ctx.enter_context(tc.tile_pool(name="tmp", bufs=4)),
            "yd": ctx.enter_context(tc.tile_pool(name="yd", bufs=2)),
            "htk": ctx.enter_context(tc.tile_pool(name="htk", bufs=2)),
            "h2n": ctx.enter_context(tc.tile_pool(name="h2n", bufs=2)),
            "psum": ctx.enter_context(tc.tile_pool(name="psum", bufs=6,
                                                   space="PSUM")),
            "psd": ctx.enter_context(tc.tile_pool(name="psd", bufs=1,
                                                  space="PSUM")),
            "pst": ctx.enter_context(tc.tile_pool(name="pst", bufs=1,
                                                  space="PSUM")),
        }
        idx_offs = np.concatenate(
            [[0], np.cumsum([int(128 * K.sum()) for K in Ks_all])])

        def load_t(t):
            Lt = int(128 * Ks_all[t].sum())
            off = int(idx_offs[t])
            idxt = pools["idx"].tile([128, Lt // 16], I16, tag="idx")
            nc.sync.dma_start(
                idxt[:], idx_d.ap()[:, off // 16 : (off + Lt) // 16])
            h1b = pools["misc"].tile([128, shp], HDT, tag="h1b")
            nc.sync.dma_start(h1b[:], h1t_d.ap()[t])
            pit = pools["misc"].tile([128, shp // 16], I16, tag="pid")
            nc.sync.dma_start(pit[:], pidx_d.ap()[t])
            return idxt, h1b, pit

        pre = load_t(0)  # before weights so the first gather starts ASAP

        # persistent weights
        wt = {}
        for nm, d in (("k1", k1_d), ("r1", r1_d), ("k2", k2_d), ("r2", r2_d)):
            for half in range(2):
                tw = pools["w"].tile([128, u4], HDT, tag=f"{nm}{half}")
                nc.sync.dma_start(tw[:], d.ap()[half * 128 : (half + 1) * 128])
                wt[f"{nm}{half}"] = tw
        b1t = pools["w"].tile([128, w * 8], F32, tag="b1")
        nc.sync.dma_start(b1t[:], b1_d.ap()[:])
        b2t = pools["w"].tile([128, 8], F32, tag="b2")
        nc.sync.dma_start(b2t[:], b2_d.ap()[:])
        wdt = pools["w"].tile([128, 2], HDT, tag="wd")
        nc.sync.dma_start(wdt[:], wd_d.ap()[:])
        bdt = pools["w"].tile([1, 1], F32, tag="bd")
        nc.sync.dma_start(bdt[:], bd_d.ap()[:])
        identt = pools["w"].tile([128, 128], HDT, tag="ident")
        nc.sync.dma_start(identt[:], ident_d.ap()[:])

        # LSTM state (fp16): h and c for both layers, [128, 2*shp]
        h1s = pools["state"].tile([128, 2 * shp], HDT, tag="h1s")
        c1s = pools["state"].tile([128, 2 * shp], HDT, tag="c1s")
        h2s = pools["state"].tile([128, 2 * shp], HDT, tag="h2s")
        c2s = pools["state"].tile([128, 2 * shp], HDT, tag="c2s")

        for t in range(w):
            idxt, h1b, pit = pre
            if t + 1 < w:
                pre = load_t(t + 1)

            h2tok = pools["htk"].tile([128, ng, f], HDT, tag="htk")

            def consume(g, gt, h2tok=h2tok, t=t):
                kg = int(Ks_all[t][g])
                # PE is idle during the t=0 prologue (no LSTM yet): let it
                # take half the tree groups there; otherwise PE is the
                # busiest engine, so trees stay on DVE.
                if t == 0 and g % 2 == 1:
                    ps = pools["pst"].tile([128, f], F32, tag="pst")
                    for j in range(kg):
                        nc.tensor.matmul(ps[:], identt[:],
                                         gt[:, j, :].bitcast(HDT),
                                         start=(j == 0), stop=(j == kg - 1))
                    nc.scalar.activation(h2tok[:, g, :], ps[:], AF.Relu)
                else:
                    _tree(nc, gt, kg)
                    nc.gpsimd.tensor_relu(h2tok[:, g, :],
                                          gt[:, 0, :].bitcast(HDT))

            _emit_mpnn(nc, pools, hsc_d, idxt, Ks_all[t], t, f, consume)

            # un-permute + transpose to [feat, node] via SBUF-source gather
            h2n = pools["h2n"].tile([128, 1, shp], HDT, tag="h2n")
            nc.gpsimd.dma_gather(
                h2n[:], h2tok[:], pit[:], shp, shp, f, transpose=True,
                sbuf_tokens_per_rank=128, sbuf_free_dim_per_rank=2 * f,
                single_packet=False)

            _lstm_step(nc, pools, (h1b[:], h2n[:, 0, :]), wt["k10"][:],
                       wt["k11"][:],
                       wt["r10"][:], wt["r11"][:], b1t[:, t * 8 : (t + 1) * 8],
                       h1s[:], c1s[:], t == 0, shp, ct)
            _lstm_step(nc, pools, (h1s[:, 0:shp], h1s[:, shp : 2 * shp]),
                       wt["k20"][:], wt["k21"][:], wt["r20"][:], wt["r21"][:],
                       b2t[:], h2s[:], c2s[:], t == 0, shp, ct)

        # dense head: y = relu(hT @ wd + bd)
        for nt in range(shp // ct):
            cs = slice(nt * ct, (nt + 1) * ct)
            ps = pools["psd"].tile([1, ct], F32, tag="psd")
            nc.tensor.matmul(ps[:], wdt[:, 0:1], h2s[:, nt * ct : (nt + 1) * ct],
                             start=True, stop=False)
            nc.tensor.matmul(ps[:], wdt[:, 1:2],
                             h2s[:, shp + nt * ct : shp + (nt + 1) * ct],
                             start=False, stop=True)
            yt = pools["yd"].tile([1, ct], F32, tag="y")
            nc.scalar.activation(yt[:], ps[:], AF.Relu, bias=bdt[:, 0:1])
            nc.sync.dma_start(y_d.ap()[:, cs], yt[:])
    nc.compile()
    return nc


# ----------------------------------------------------------------- kernel()

def kernel(**inputs):
    X = np.asarray(inputs["X"], np.float32)
    edge_src = np.asarray(inputs["edge_src"])
    edge_dst = np.asarray(inputs["edge_dst"])
    w, n, f = X.shape
    u4 = int(np.asarray(inputs["k1"]).shape[1])
    sh = n // NCORES
    ng = max(1, (sh + 127) // 128)
    shp = ng * 128
    nrows = n + 1  # one zero pad row for padded gather slots
    pad_tok = n
    ct = min(512, shp)
    assert shp % ct == 0

    # fold BN params
    rsg1 = (np.asarray(inputs["gamma1"], np.float32)
            / np.sqrt(np.asarray(inputs["var1"], np.float32) + EPS))
    bet1 = (np.asarray(inputs["beta1"], np.float32)
            - np.asarray(inputs["mean1"], np.float32) * rsg1)
    rsg2 = (np.asarray(inputs["gamma2"], np.float32)
            / np.sqrt(np.asarray(inputs["var2"], np.float32) + EPS))
    bet2 = (np.asarray(inputs["beta2"], np.float32)
            - np.asarray(inputs["mean2"], np.float32) * rsg2)

    # edge plans
    Ks_all, streams_all, cnts_all, perms_all = [], [], [], []
    for t in range(w):
        K, streams, cnts, perms = _plan_t(np.asarray(edge_src[t]),
                                          np.asarray(edge_dst[t]),
                                          n, NCORES, shp, pad_tok)
        Ks_all.append(K)
        streams_all.append(streams)
        cnts_all.append(cnts)
        perms_all.append(perms)

    # packed inputs for launch A (fp16 rows viewed as uint32 for the gather)
    xs = np.zeros((w, nrows, f), np.float16)
    xs[:, :n] = (X * rsg1[:, None, :]).astype(np.float16)
    xs_u32 = np.ascontiguousarray(xs).view(np.uint32)
    idx_packed = []
    for c in range(NCORES):
        idx_packed.append(np.concatenate(
            [_pack_idx_blocks(streams_all[t][c], Ks_all[t]) for t in range(w)],
            axis=1))
    ident = np.eye(128, dtype=np.float16)

    # ---- launch A
    nc_a = _build_launch_a(Ks_all, w, f, nrows, shp)
    in_maps_a = [
        dict(xs=xs_u32, idx=idx_packed[c], ident=ident)
        for c in range(NCORES)
    ]
    LAST_STATS["nc_a"] = nc_a
    res_a = run_bass_kernel_spmd(nc_a, in_maps_a, core_ids=list(range(NCORES)),
                                 trace=PROFILE)
    LAST_STATS["a_exec_ns"] = res_a.exec_time_ns

    # ---- host exchange: decode token layout, unpermute rows, apply the
    # mean scale (1/cnt) and bet1, rescale by rsg2 for the mpnn2 gather
    h1_full = np.empty((w, n, f), np.float32)
    for c in range(NCORES):
        shard = res_a.results[c]["h1"].reshape(w, 128, ng, f)
        shard = shard.transpose(0, 2, 1, 3).reshape(w, shp, f)  # [w, pos, f]
        for t in range(w):
            alpha = 1.0 / np.maximum(cnts_all[t][c][perms_all[t][c]], 1.0)
            h1_full[t, c * sh + perms_all[t][c], :] = (
                shard[t, :sh].astype(np.float32) * alpha[:sh, None] + bet1[t])
    hsc = np.zeros((w, nrows, f), np.float16)
    hsc[:, :n] = (h1_full * rsg2[:, None, :]).astype(np.float16)
    hsc_u32 = np.ascontiguousarray(hsc).view(np.uint32)
    h1t = []
    for c in range(NCORES):
        v = np.zeros((w, 128, shp), np.float16)
        v[:, :, :sh] = h1_full[:, c * sh : (c + 1) * sh, :].transpose(0, 2, 1)
        h1t.append(v)
    # inverse-permutation gather indices for launch B's h2 un-permute
    pinv_packed = []
    for c in range(NCORES):
        blocks = []
        for t in range(w):
            pos_of = np.zeros(shp, np.int64)
            pos_of[perms_all[t][c]] = np.arange(sh)
            blocks.append(_pack_idx_blocks(pos_of, [shp // 128]))
        pinv_packed.append(np.stack(blocks))

    # ---- launch B
    k1 = np.asarray(inputs["k1"], np.float32).astype(np.float16)
    r1 = np.asarray(inputs["r1"], np.float32).astype(np.float16)
    k2 = np.asarray(inputs["k2"], np.float32).astype(np.float16)
    r2 = np.asarray(inputs["r2"], np.float32).astype(np.float16)
    # bet2 is constant across nodes: fold its k1-contribution into b1,
    # per timestep (bet2 varies with t)
    b1_cols = []
    for t in range(w):
        b1_t = (np.asarray(inputs["b1"], np.float32)
                + bet2[t] @ np.asarray(inputs["k1"], np.float32)[f:, :])
        b1_cols.append(b1_t.reshape(8, 128).T)
    b1_all = np.ascontiguousarray(np.concatenate(b1_cols, axis=1))  # [128,w*8]
    b2 = np.asarray(inputs["b2"], np.float32).reshape(8, 128).T.copy()
    wd = np.asarray(inputs["wd"], np.float32).reshape(2, 128).T.copy().astype(
        np.float16)
    bd = np.asarray(inputs["bd"], np.float32).reshape(1, 1)

    nc_b = _build_launch_b(Ks_all, w, f, nrows, shp, u4)
    in_maps_b = [
        dict(hsc=hsc_u32, idx=idx_packed[c], h1t=h1t[c],
             k1=k1, r1=r1, k2=k2, r2=r2,
             b1=b1_all, b2=b2, wd=wd,
             bd=bd, pinv=pinv_packed[c], ident=ident)
        for c in range(NCORES)
    ]
    LAST_STATS["nc_b"] = nc_b
    res_b = run_bass_kernel_spmd(nc_b, in_maps_b, core_ids=list(range(NCORES)),
                                 trace=PROFILE)
    LAST_STATS["b_exec_ns"] = res_b.exec_time_ns

    out = np.empty((n, 1), np.float32)
    for c in range(NCORES):
        out[c * sh : (c + 1) * sh, 0] = res_b.results[c]["y"][0, :sh]
    return out


# revision 39
# speedup vs baseline: 1.0022x; 1.0022x over previous
"""Trainium2 Bass kernel for nn_Net_84782654423525 (GNN message passing + LSTM).

Strategy (8 NeuronCores, dst-sharded nodes):
  Launch A (mpnn1): per core, HBM-source dma_gather of rsg1-prescaled X
    rows for edges whose dst it owns. Rows are fp16 [128] viewed as
    uint32 [64] (elem_size=64: halves the per-descriptor DMA cost, which
    scales with element count, not bytes). transpose=False output is
    [128 dst, K slots, 64] per degree-sorted 128-node group with fixed
    padded K. Slot-sum split across the otherwise-idle PE (identity-
    matmul PSUM accumulate) and DVE (fp16 tree adds); relu on Act ->
    h1 shard (token layout, fp16) to HBM.
  Host: reassemble full h1 (unpermute degree-sorted rows), apply the
    mean 1/cnt scale and bet1, build the rsg2-prescaled uint32 gather
    source for mpnn2; fold bet2 into per-timestep LSTM bias b1.
  Launch B (mpnn2 + 2-layer LSTM + dense): same uint32 HBM gathers ->
    DVE tree-sum (PE assists during the t=0 prologue) -> relu on GPSIMD
    -> h2 token tile; un-permute + transpose via one SBUF-source gather;
    LSTM over [h1;h2] with fp16 matmuls (features-on-partitions), fp32
    PSUM, gate activations on Act, the two independent elementwise muls
    on GPSIMD and the c-update chain on DVE; final dense + ReLU.

Engine balance per launch (cost model): A ~226us = max(DMA gathers ~213,
PE+DVE split tree ~110 each); B ~461us = max(PE 399 LSTM, Act 376 gates,
DVE 350 tree+chain, Pool 340 descgen+relu/mul offload).
"""

import os
import sys
from contextlib import ExitStack

import numpy as np

sys.path.insert(0, "/opt/trn_rl_repo")

import concourse.bacc as bacc
import concourse.tile as tile
from concourse import mybir
from concourse.bass_utils import run_bass_kernel_spmd

HDT = mybir.dt.float16
F32 = mybir.dt.float32
I16 = mybir.dt.int16
U32 = mybir.dt.uint32
AF = mybir.ActivationFunctionType
EPS = 1e-3
NCORES = 8

PROFILE = bool(int(os.environ.get("KERNEL_PROFILE", "0")))
LAST_STATS = {}

try:  # trace=True requires antenv.axon_hooks; fall back gracefully
    from antenv.axon_hooks import get_axon_ntff_profile_hook  # noqa: F401
except Exception:
    PROFILE = False


# ---------------------------------------------------------------- host prep

def _pack_idx_blocks(stream, ks):
    """Per-group idx blocks packed for dma_gather: idx i of a block lives at
    [i % 16, i // 16]; blocks concatenated along cols; tiled to 128 rows."""
    blocks = []
    off = 0
    for k in ks:
        n = 128 * int(k)
        s = stream[off : off + n]
        blocks.append(s.reshape(n // 16, 16).T)
        off += n
    m = np.concatenate(blocks, axis=1)  # [16, L/16]
    return np.ascontiguousarray(np.tile(m, (8, 1))).astype(np.int16)


def _plan_t(src, dst, n, ncores, shp, pad_tok):
    """Edge plan for one timestep with degree-sorted node groups (tight K).

    Nodes of each core's shard are permuted into degree-descending order so
    that per-group max degree (the padded slot count K) is near the group's
    degree quantile across all cores. Returns (K[NG], streams per core,
    cnt per core (permuted order), perm per core)."""
    sh = n // ncores
    ng = shp // 128
    per_core = []
    for c in range(ncores):
        m = (dst >= c * sh) & (dst < (c + 1) * sh)
        dl = (dst[m] - c * sh).astype(np.int64)
        sl = src[m].astype(np.int64)
        order = np.argsort(dl, kind="stable")
        dl = dl[order]
        sl = sl[order]
        cnt = np.bincount(dl, minlength=sh)
        perm = np.argsort(-cnt, kind="stable")  # natural ids, deg-desc order
        pos_of = np.empty(sh, np.int64)
        pos_of[perm] = np.arange(sh)
        per_core.append((dl, sl, cnt, perm, pos_of))
    K = np.full(ng, 2, np.int64)
    for dl, sl, cnt, perm, pos_of in per_core:
        cp = np.zeros(shp, np.int64)
        cp[:sh] = cnt[perm]
        K = np.maximum(K, cp.reshape(ng, 128).max(1))
    K = np.maximum(K, 2)
    base = np.concatenate([[0], np.cumsum(128 * K)])
    L = int(base[-1])
    streams, cnts, perms = [], [], []
    for dl, sl, cnt, perm, pos_of in per_core:
        stream = np.full(L, pad_tok, np.int64)
        starts = np.concatenate([[0], np.cumsum(cnt)])
        j = np.arange(dl.size) - starts[dl]
        p = pos_of[dl]  # permuted position of each edge's dst
        pos = base[p // 128] + j * 128 + (p % 128)
        stream[pos] = sl
        streams.append(stream)
        cnts.append(cnt)
        perms.append(perm)
    return K, streams, cnts, perms


# ---------------------------------------------------------- device builders

def _tree(nc, gt, k):
    """In-place slot-dim tree sum of gt (uint32 tile [128, k, f//2], treated
    as fp16 [128, k, f]); result in gt[:, 0, :].bitcast(f16)."""
    v = lambda a, b: gt[:, a:b, :].bitcast(HDT)
    cur = k
    while cur > 1:
        if cur % 2:
            nc.vector.tensor_add(v(0, 1), v(0, 1), v(cur - 1, cur))
            cur -= 1
        h = cur // 2
        nc.vector.tensor_add(v(0, h), v(0, h), v(h, cur))
        cur = h


def _emit_mpnn(nc, pools, src_d, idxt, Ks, t, f, consume):
    """HBM-source uint32 gather (elem=f//2 x 4B = one fp16 feature row per
    index) for one timestep. Calls consume(g, gt) per group right after its
    gather; gt is uint32 [128, kg, f//2] holding fp16 rows."""
    ng = len(Ks)
    e4 = f // 2  # uint32 elements per fp16 feature row
    goff = 0
    for g in range(ng):
        kg = int(Ks[g])
        gt = pools["g"].tile([128, kg, e4], U32, tag="g")
        nc.gpsimd.dma_gather(
            gt[:],
            src_d.ap()[t],
            idxt[:, goff // 16 : (goff + 128 * kg) // 16],
            128 * kg,
            128 * kg,
            e4,
            transpose=False,
            single_packet=False,
        )
        goff += 128 * kg
        consume(g, gt)


def _build_launch_a(Ks_all, w, f, nrows, shp):
    ng = shp // 128
    nc = bacc.Bacc("TRN2", target_bir_lowering=False, debug=False,
                   num_devices=NCORES)
    Ltot = int(sum(128 * K.sum() for K in Ks_all))
    xs_d = nc.dram_tensor("xs", [w, nrows, f // 2], U32, kind="ExternalInput")
    idx_d = nc.dram_tensor("idx", [128, Ltot // 16], I16, kind="ExternalInput")
    ident_d = nc.dram_tensor("ident", [128, 128], HDT, kind="ExternalInput")
    h1_d = nc.dram_tensor("h1", [w, 128, ng * f], HDT, kind="ExternalOutput")

    with tile.TileContext(nc) as tc, ExitStack() as ctx, \
            nc.allow_low_precision(reason="fp16 sums by design"):
        pools = {
            "idx": ctx.enter_context(tc.tile_pool(name="idx", bufs=2)),
            "g": ctx.enter_context(tc.tile_pool(name="g", bufs=5)),
            "hs": ctx.enter_context(tc.tile_pool(name="hs", bufs=2)),
            "w": ctx.enter_context(tc.tile_pool(name="w", bufs=1)),
            "psum": ctx.enter_context(tc.tile_pool(name="psum", bufs=6,
                                                   space="PSUM")),
        }
        identt_a = pools["w"].tile([128, 128], HDT, tag="ident")
        nc.sync.dma_start(identt_a[:], ident_d.ap()[:])
        idx_off = 0
        for t in range(w):
            Lt = int(128 * Ks_all[t].sum())
            idxt = pools["idx"].tile([128, Lt // 16], I16, tag="idx")
            nc.sync.dma_start(
                idxt[:], idx_d.ap()[:, idx_off // 16 : (idx_off + Lt) // 16])
            hs = pools["hs"].tile([128, ng, f], HDT, tag="hs")

            def consume(g, gt, hs=hs, t=t):
                # slot-sum split across the otherwise-idle PE and DVE
                kg = int(Ks_all[t][g])
                if g % 2 == 0:
                    ps = pools["psum"].tile([128, f], F32, tag="ps")
                    for j in range(kg):
                        nc.tensor.matmul(ps[:], identt_a[:],
                                         gt[:, j, :].bitcast(HDT),
                                         start=(j == 0), stop=(j == kg - 1))
                    nc.scalar.activation(hs[:, g, :], ps[:], AF.Relu)
                else:
                    _tree(nc, gt, kg)
                    nc.scalar.activation(hs[:, g, :],
                                         gt[:, 0, :].bitcast(HDT), AF.Relu)

            _emit_mpnn(nc, pools, xs_d, idxt, Ks_all[t], t, f, consume)
            idx_off += Lt
            nc.sync.dma_start(h1_d.ap()[t], hs[:])
    nc.compile()
    return nc


def _lstm_step(nc, pools, xc, ka, kb, ra, rb_, btile, hbf, cst, first, shp, ct):
    """One LSTM step. xc: (chunk0, chunk1) fp16 [128, shp] APs of x^T.
    hbf/cst: [128, 2*shp] fp16 tiles (h, c), updated in place."""
    ntile = shp // ct
    for nt in range(ntile):
        cs = slice(nt * ct, (nt + 1) * ct)
        gates = []
        for gc in range(8):
            gs = slice(gc * 128, (gc + 1) * 128)
            ps = pools["psum"].tile([128, ct], F32, tag="ps")
            nc.tensor.matmul(ps[:], ka[:, gs], xc[0][:, cs], start=True,
                             stop=False)
            nc.tensor.matmul(ps[:], kb[:, gs], xc[1][:, cs], start=False,
                             stop=first)
            if not first:
                nc.tensor.matmul(ps[:], ra[:, gs], hbf[:, nt * ct : (nt + 1) * ct],
                                 start=False, stop=False)
                nc.tensor.matmul(ps[:], rb_[:, gs],
                                 hbf[:, shp + nt * ct : shp + (nt + 1) * ct],
                                 start=False, stop=True)
            gt_ = pools["gate"].tile([128, ct], HDT, tag="gate")
            func = AF.Tanh if gc in (4, 5) else AF.Sigmoid
            nc.scalar.activation(gt_[:], ps[:], func, bias=btile[:, gc : gc + 1])
            gates.append(gt_)
        for uc in range(2):
            i_, f_, g_, o_ = gates[0 + uc], gates[2 + uc], gates[4 + uc], gates[6 + uc]
            csl = cst[:, uc * shp + nt * ct : uc * shp + (nt + 1) * ct]
            tmp = pools["tmp"].tile([128, ct], HDT, tag="tmp")
            # the two independent muls go to GPSIMD to relieve DVE; the
            # c-chain (f*c, +) stays on DVE for latency
            nc.gpsimd.tensor_mul(tmp[:], i_[:], g_[:])
            if first:
                nc.vector.tensor_copy(csl, tmp[:])
            else:
                nc.vector.tensor_mul(csl, f_[:], csl)
                nc.vector.tensor_add(csl, csl, tmp[:])
            th = pools["tmp"].tile([128, ct], HDT, tag="th")
            nc.scalar.activation(th[:], csl, AF.Tanh)
            nc.gpsimd.tensor_mul(hbf[:, uc * shp + nt * ct : uc * shp + (nt + 1) * ct],
                                 o_[:], th[:])


def _build_launch_b(Ks_all, w, f, nrows, shp, u4):
    ng = shp // 128
    nc = bacc.Bacc("TRN2", target_bir_lowering=False, debug=False,
                   num_devices=NCORES)
    Ltot = int(sum(128 * K.sum() for K in Ks_all))
    ct = min(512, shp)
    hsc_d = nc.dram_tensor("hsc", [w, nrows, f // 2], U32, kind="ExternalInput")
    idx_d = nc.dram_tensor("idx", [128, Ltot // 16], I16, kind="ExternalInput")
    h1t_d = nc.dram_tensor("h1t", [w, 128, shp], HDT, kind="ExternalInput")
    k1_d = nc.dram_tensor("k1", [256, u4], HDT, kind="ExternalInput")
    r1_d = nc.dram_tensor("r1", [256, u4], HDT, kind="ExternalInput")
    k2_d = nc.dram_tensor("k2", [256, u4], HDT, kind="ExternalInput")
    r2_d = nc.dram_tensor("r2", [256, u4], HDT, kind="ExternalInput")
    ident_d = nc.dram_tensor("ident", [128, 128], HDT, kind="ExternalInput")
    b1_d = nc.dram_tensor("b1", [128, w * 8], F32, kind="ExternalInput")
    b2_d = nc.dram_tensor("b2", [128, 8], F32, kind="ExternalInput")
    wd_d = nc.dram_tensor("wd", [128, 2], HDT, kind="ExternalInput")
    bd_d = nc.dram_tensor("bd", [1, 1], F32, kind="ExternalInput")
    pidx_d = nc.dram_tensor("pinv", [w, 128, shp // 16], I16,
                            kind="ExternalInput")
    y_d = nc.dram_tensor("y", [1, shp], F32, kind="ExternalOutput")

    with tile.TileContext(nc) as tc, ExitStack() as ctx, \
            nc.allow_low_precision(reason="fp16 state/tree by design"):
        pools = {
            "idx": ctx.enter_context(tc.tile_pool(name="idx", bufs=2)),
            "g": ctx.enter_context(tc.tile_pool(name="g", bufs=5)),
            "misc": ctx.enter_context(tc.tile_pool(name="misc", bufs=2)),
            "w": ctx.enter_context(tc.tile_pool(name="w", bufs=1)),
            "state": ctx.enter_context(tc.tile_pool(name="state", bufs=1)),
            "gate": ctx.enter_context(tc.tile_pool(name="gate", bufs=10)),
            "tmp": ctx.enter_context(tc.tile_pool(name="tmp", bufs=4)),
            "yd": ctx.enter_context(tc.tile_pool(name="yd", bufs=2)),
            "htk": ctx.enter_context(tc.tile_pool(name="htk", bufs=2)),
            "h2n": ctx.enter_context(tc.tile_pool(name="h2n", bufs=2)),
            "psum": ctx.enter_context(tc.tile_pool(name="psum", bufs=6,
                                                   space="PSUM")),
            "psd": ctx.enter_context(tc.tile_pool(name="psd", bufs=1,
                                                  space="PSUM")),
            "pst": ctx.enter_context(tc.tile_pool(name="pst", bufs=1,
                                                  space="PSUM")),
        }
        idx_offs = np.concatenate(
            [[0], np.cumsum([int(128 * K.sum()) for K in Ks_all])])

        def load_t(t):
            Lt = int(128 * Ks_all[t].sum())
            off = int(idx_offs[t])
            idxt = pools["idx"].tile([128, Lt // 16], I16, tag="idx")
            nc.sync.dma_start(
                idxt[:], idx_d.ap()[:, off // 16 : (off + Lt) // 16])
            h1b = pools["misc"].tile([128, shp], HDT, tag="h1b")
            nc.sync.dma_start(h1b[:], h1t_d.ap()[t])
            pit = pools["misc"].tile([128, shp // 16], I16, tag="pid")
            nc.sync.dma_start(pit[:], pidx_d.ap()[t])
            return idxt, h1b, pit

        pre = load_t(0)  # before weights so the first gather starts ASAP

        # persistent weights
        wt = {}
        for nm, d in (("k1", k1_d), ("r1", r1_d), ("k2", k2_d), ("r2", r2_d)):
            for half in range(2):
                tw = pools["w"].tile([128, u4], HDT, tag=f"{nm}{half}")
                nc.sync.dma_start(tw[:], d.ap()[half * 128 : (half + 1) * 128])
                wt[f"{nm}{half}"] = tw
        b1t = pools["w"].tile([128, w * 8], F32, tag="b1")
        nc.sync.dma_start(b1t[:], b1_d.ap()[:])
        b2t = pools["w"].tile([128, 8], F32, tag="b2")
        nc.sync.dma_start(b2t[:], b2_d.ap()[:])
        wdt = pools["w"].tile([128, 2], HDT, tag="wd")
        nc.sync.dma_start(wdt[:], wd_d.ap()[:])
        bdt = pools["w"].tile([1, 1], F32, tag="bd")
        nc.sync.dma_start(bdt[:], bd_d.ap()[:])
        identt = pools["w"].tile([128, 128], HDT, tag="ident")
        nc.sync.dma_start(identt[:], ident_d.ap()[:])

        # LSTM state (fp16): h and c for both layers, [128, 2*shp]
        h1s = pools["state"].tile([128, 2 * shp], HDT, tag="h1s")
        c1s = pools["state"].tile([128, 2 * shp], HDT, tag="c1s")
        h2s = pools["state"].tile([128, 2 * shp], HDT, tag="h2s")
        c2s = pools["state"].tile([128, 2 * shp], HDT, tag="c2s")

        for t in range(w):
            idxt, h1b, pit = pre
            if t + 1 < w:
                pre = load_t(t + 1)

            h2tok = pools["htk"].tile([128, ng, f], HDT, tag="htk")

            def consume(g, gt, h2tok=h2tok, t=t):
                kg = int(Ks_all[t][g])
                # PE is idle during the t=0 prologue (no LSTM yet): let it
                # take half the tree groups there; otherwise PE is the
                # busiest engine, so trees stay on DVE.
                if t == 0 and g % 2 == 1:
                    ps = pools["pst"].tile([128, f], F32, tag="pst")
                    for j in range(kg):
                        nc.tensor.matmul(ps[:], identt[:],
                                         gt[:, j, :].bitcast(HDT),
                                         start=(j == 0), stop=(j == kg - 1))
                    nc.scalar.activation(h2tok[:, g, :], ps[:], AF.Relu)
                else:
                    _tree(nc, gt, kg)
                    nc.gpsimd.tensor_relu(h2tok[:, g, :],
                                          gt[:, 0, :].bitcast(HDT))

            _emit_mpnn(nc, pools, hsc_d, idxt, Ks_all[t], t, f, consume)

            # un-permute + transpose to [feat, node] via SBUF-source gather
            h2n = pools["h2n"].tile([128, 1, shp], HDT, tag="h2n")
            nc.gpsimd.dma_gather(
                h2n[:], h2tok[:], pit[:], shp, shp, f, transpose=True,
                sbuf_tokens_per_rank=128, sbuf_free_dim_per_rank=2 * f,
                single_packet=False)

            _lstm_step(nc, pools, (h1b[:], h2n[:, 0, :]), wt["k10"][:],
                       wt["k11"][:],
                       wt["r10"][:], wt["r11"][:], b1t[:, t * 8 : (t + 1) * 8],
                       h1s[:], c1s[:], t == 0, shp, ct)
            _lstm_step(nc, pools, (h1s[:, 0:shp], h1s[:, shp : 2 * shp]),
                       wt["k20"][:], wt["k21"][:], wt["r20"][:], wt["r21"][:],
                       b2t[:], h2s[:], c2s[:], t == 0, shp, ct)

        # dense head: y = relu(hT @ wd + bd)
        for nt in range(shp // ct):
            cs = slice(nt * ct, (nt + 1) * ct)
            ps = pools["psd"].tile([1, ct], F32, tag="psd")
            nc.tensor.matmul(ps[:], wdt[:, 0:1], h2s[:, nt * ct : (nt + 1) * ct],
                             start=True, stop=False)
            nc.tensor.matmul(ps[:], wdt[:, 1:2],
                             h2s[:, shp + nt * ct : shp + (nt + 1) * ct],
                             start=False, stop=True)
            yt = pools["yd"].tile([1, ct], F32, tag="y")
            nc.scalar.activation(yt[:], ps[:], AF.Relu, bias=bdt[:, 0:1])
            nc.sync.dma_start(y_d.ap()[:, cs], yt[:])
    nc.compile()
    return nc


# ----------------------------------------------------------------- kernel()

def kernel(**inputs):
    X = np.asarray(inputs["X"], np.float32)
    edge_src = np.asarray(inputs["edge_src"])
    edge_dst = np.asarray(inputs["edge_dst"])
    w, n, f = X.shape
    u4 = int(np.asarray(inputs["k1"]).shape[1])
    sh = n // NCORES
    ng = max(1, (sh + 127) // 128)
    shp = ng * 128
    nrows = n + 1  # one zero pad row for padded gather slots
    pad_tok = n
    ct = min(512, shp)
    assert shp % ct == 0

    # fold BN params
    rsg1 = (np.asarray(inputs["gamma1"], np.float32)
            / np.sqrt(np.asarray(inputs["var1"], np.float32) + EPS))
    bet1 = (np.asarray(inputs["beta1"], np.float32)
            - np.asarray(inputs["mean1"], np.float32) * rsg1)
    rsg2 = (np.asarray(inputs["gamma2"], np.float32)
            / np.sqrt(np.asarray(inputs["var2"], np.float32) + EPS))
    bet2 = (np.asarray(inputs["beta2"], np.float32)
            - np.asarray(inputs["mean2"], np.float32) * rsg2)

    # edge plans
    Ks_all, streams_all, cnts_all, perms_all = [], [], [], []
    for t in range(w):
        K, streams, cnts, perms = _plan_t(np.asarray(edge_src[t]),
                                          np.asarray(edge_dst[t]),
                                          n, NCORES, shp, pad_tok)
        Ks_all.append(K)
        streams_all.append(streams)
        cnts_all.append(cnts)
        perms_all.append(perms)

    # packed inputs for launch A (fp16 rows viewed as uint32 for the gather)
    xs = np.zeros((w, nrows, f), np.float16)
    xs[:, :n] = (X * rsg1[:, None, :]).astype(np.float16)
    xs_u32 = np.ascontiguousarray(xs).view(np.uint32)
    idx_packed = []
    for c in range(NCORES):
        idx_packed.append(np.concatenate(
            [_pack_idx_blocks(streams_all[t][c], Ks_all[t]) for t in range(w)],
            axis=1))
    ident = np.eye(128, dtype=np.float16)

    # ---- launch A
    nc_a = _build_launch_a(Ks_all, w, f, nrows, shp)
    in_maps_a = [
        dict(xs=xs_u32, idx=idx_packed[c], ident=ident)
        for c in range(NCORES)
    ]
    LAST_STATS["nc_a"] = nc_a
    res_a = run_bass_kernel_spmd(nc_a, in_maps_a, core_ids=list(range(NCORES)),
                                 trace=PROFILE)
    LAST_STATS["a_exec_ns"] = res_a.exec_time_ns

    # ---- host exchange: decode token layout, unpermute rows, apply the
    # mean scale (1/cnt) and bet1, rescale by rsg2 for the mpnn2 gather
    h1_full = np.empty((w, n, f), np.float32)
    for c in range(NCORES):
        shard = res_a.results[c]["h1"].reshape(w, 128, ng, f)
        shard = shard.transpose(0, 2, 1, 3).reshape(w, shp, f)  # [w, pos, f]
        for t in range(w):
            alpha = 1.0 / np.maximum(cnts_all[t][c][perms_all[t][c]], 1.0)
            h1_full[t, c * sh + perms_all[t][c], :] = (
                shard[t, :sh].astype(np.float32) * alpha[:sh, None] + bet1[t])
    hsc = np.zeros((w, nrows, f), np.float16)
    hsc[:, :n] = (h1_full * rsg2[:, None, :]).astype(np.float16)
    hsc_u32 = np.ascontiguousarray(hsc).view(np.uint32)
    h1t = []
    for c in range(NCORES):
        v = np.zeros((w, 128, shp), np.float16)
        v[:, :, :sh] = h1_full[:, c * sh : (c + 1) * sh, :].transpose(0, 2, 1)
        h1t.append(v)
    # inverse-permutation gather indices for launch B's h2 un-permute
    pinv_packed = []
    for c in range(NCORES):
        blocks = []
        for t in range(w):
            pos_of = np.zeros(shp, np.int64)
            pos_of[perms_all[t][c]] = np.arange(sh)
            blocks.append(_pack_idx_blocks(pos_of, [shp // 128]))
        pinv_packed.append(np.stack(blocks))

    # ---- launch B
    k1 = np.asarray(inputs["k1"], np.float32).astype(np.float16)
    r1 = np.asarray(inputs["r1"], np.float32).astype(np.float16)
    k2 = np.asarray(inputs["k2"], np.float32).astype(np.float16)
    r2 = np.asarray(inputs["r2"], np.float32).astype(np.float16)
    # bet2 is constant across nodes: fold its k1-contribution into b1,
    # per timestep (bet2 varies with t)
    b1_cols = []
    for t in range(w):
        b1_t = (np.asarray(inputs["b1"], np.float32)
                + bet2[t] @ np.asarray(inputs["k1"], np.float32)[f:, :])
        b1_cols.append(b1_t.reshape(8, 128).T)
    b1_all = np.ascontiguousarray(np.concatenate(b1_cols, axis=1))  # [128,w*8]
    b2 = np.asarray(inputs["b2"], np.float32).reshape(8, 128).T.copy()
    wd = np.asarray(inputs["wd"], np.float32).reshape(2, 128).T.copy().astype(
        np.float16)
    bd = np.asarray(inputs["bd"], np.float32).reshape(1, 1)

    nc_b = _build_launch_b(Ks_all, w, f, nrows, shp, u4)
    in_maps_b = [
        dict(hsc=hsc_u32, idx=idx_packed[c], h1t=h1t[c],
             k1=k1, r1=r1, k2=k2, r2=r2,
             b1=b1_all, b2=b2, wd=wd,
             bd=bd, pinv=pinv_packed[c], ident=ident)
        for c in range(NCORES)
    ]
    LAST_STATS["nc_b"] = nc_b
    res_b = run_bass_kernel_spmd(nc_b, in_maps_b, core_ids=list(range(NCORES)),
                                 trace=PROFILE)
    LAST_STATS["b_exec_ns"] = res_b.exec_time_ns

    out = np.empty((n, 1), np.float32)
    for c in range(NCORES):
        out[c * sh : (c + 1) * sh, 0] = res_b.results[c]["y"][0, :sh]
    return out
